# revision 10
# baseline (speedup 1.0000x reference)
"""Trainium2 Bass kernel for the attention-LSTM decoder (nn_Decoder).

Strategy (8 NeuronCores):
  - Attention batch-sharded: each core owns B/8 = 4 batches (enc_out slice,
    encW2 precompute, tanh energies, softmax, context).
  - LSTM tensor-parallel over the hidden dim: each core owns a 128-wide
    hidden slice -> 512 gate rows per layer; h slices are AllGathered
    (feature-major) each step.
  - Vocab projection tensor-parallel over V: deferred to one big matmul over
    all (step, batch) samples at the end; global logsumexp via one AllReduce.
Dtypes: bf16 storage for big operands, fp32 PSUM accumulation everywhere,
fp32 cell state, fp32r for the embedding table.
"""
import os
import sys

sys.path.insert(0, "/opt/trn_rl_repo")

import numpy as np
import ml_dtypes

import concourse.bass as bass
import concourse.bacc as bacc
import concourse.mybir as mybir
import concourse.tile as tile
from concourse import bass_utils
from concourse.masks import make_identity

BF = ml_dtypes.bfloat16
dt = mybir.dt
AFT = mybir.ActivationFunctionType
ALU = mybir.AluOpType

B, T, H, E, V, S = 32, 512, 1024, 300, 32000, 50
NCORES = 8
BPC = B // NCORES      # 4 batches per core
GS = H // NCORES       # 128-wide hidden slice per core
NG = 4 * GS            # 512 gate rows per core
VPC = V // NCORES      # 4000 vocab rows per core
EP = 384               # padded embedding feature dim (3 k-tiles)
KE = EP // 128         # 3
KH = H // 128          # 8
KT = T // 128          # 4
NSAMP = S * B          # 1600
S_EFF = int(os.environ.get("DECODER_STEPS", str(S)))
RG = [list(range(NCORES))]

# phase-4 vocab chunking
VCH = [512] * 7 + [416]
VOFF = [sum(VCH[:i]) for i in range(len(VCH))]
MTILES = [(m * 128, min(128, NSAMP - m * 128)) for m in range((NSAMP + 127) // 128)]


def build(nc):
    di = {}

    def inp(name, shape, dtype):
        di[name] = nc.dram_tensor(name, list(shape), dtype, kind="ExternalInput")
        return di[name]

    enc_nat = inp("enc_nat", (BPC, T, H), dt.bfloat16)
    enc_tr = inp("enc_tr", (BPC, H, T), dt.bfloat16)
    w2t = inp("w2t", (H, H), dt.bfloat16)
    w1t = inp("w1t", (H, H), dt.bfloat16)
    attn_bias = inp("attn_bias", (1, H), dt.bfloat16)
    vvec = inp("vvec", (H, 1), dt.bfloat16)
    emb_tab = inp("emb_tab", (V, E), dt.float32)
    qidx = inp("qidx", (NSAMP, 1), dt.int32)
    wih0e = inp("wih0e", (EP, NG), dt.bfloat16)
    wih0c = inp("wih0c", (H, NG), dt.bfloat16)
    whh0 = inp("whh0", (H, NG), dt.bfloat16)
    wih1 = inp("wih1", (H, NG), dt.bfloat16)
    whh1 = inp("whh1", (H, NG), dt.bfloat16)
    bias_g0 = inp("bias_g0", (1, NG), dt.bfloat16)
    bias_g1 = inp("bias_g1", (1, NG), dt.bfloat16)
    sel = inp("sel", (B, BPC), dt.bfloat16)
    h0t_init = inp("h0t_init", (H, B), dt.bfloat16)
    h1t_init = inp("h1t_init", (H, B), dt.bfloat16)
    c0_l0 = inp("c0_l0", (B, GS), dt.float32)
    c0_l1 = inp("c0_l1", (B, GS), dt.float32)
    genw_t = inp("genw_t", (H, VPC), dt.bfloat16)
    genb_v = inp("genb_v", (1, VPC), dt.bfloat16)
    logp = nc.dram_tensor("logp", [NSAMP, VPC], dt.float32, kind="ExternalOutput")

    with tile.TileContext(nc) as tc:
        _body(nc, tc, di, logp)
    return di


def _body(nc, tc, di, logp):
    glob_cm = tc.tile_pool(name="glob", bufs=1)
    glob = glob_cm.__enter__()
    dram_cm = tc.tile_pool(name="dram", bufs=1, space="DRAM")
    dram = dram_cm.__enter__()

    # ---- global constants ----
    id_bf = glob.tile([128, 128], dt.bfloat16, name="id_bf")
    id_f32 = glob.tile([128, 128], dt.float32, name="id_f32")
    make_identity(nc, id_bf[:])
    make_identity(nc, id_f32[:])
    ones_bf = glob.tile([1, 128], dt.bfloat16, name="ones_bf")
    nc.gpsimd.memset(ones_bf[:], 1.0)

    # h1T history lives in shared DRAM: tile s = h1T after step s
    hall_steps = [
        dram.tile([NCORES * GS, B], dt.bfloat16, name=f"hall{s}",
                  addr_space="Shared")
        for s in range(S)
    ]
    sume_all = glob.tile([128, len(MTILES)], dt.float32, name="sume_all")

    # ---------------- phase 0/1: loop-scoped persistent tensors ----------------
    loopers_cm = tc.tile_pool(name="loopers", bufs=1)
    loopers = loopers_cm.__enter__()

    enc_nat_sb = loopers.tile([128, BPC, KT, H], dt.bfloat16, name="enc_nat_sb")
    for _b in range(BPC):
        nc.sync.dma_start(
            enc_nat_sb[:, _b, :, :],
            di["enc_nat"].ap()[_b].rearrange("(k p) h -> p k h", p=128),
        )
    w1t_sb = loopers.tile([128, KH, H], dt.bfloat16, name="w1t_sb")
    nc.sync.dma_start(w1t_sb[:], di["w1t"].ap().rearrange("(k p) h -> p k h", p=128))
    wih0e_sb = loopers.tile([128, KE, NG], dt.bfloat16, name="wih0e_sb")
    nc.sync.dma_start(wih0e_sb[:], di["wih0e"].ap().rearrange("(k p) g -> p k g", p=128))
    wih0c_sb = loopers.tile([128, KH, NG], dt.bfloat16, name="wih0c_sb")
    nc.sync.dma_start(wih0c_sb[:], di["wih0c"].ap().rearrange("(k p) g -> p k g", p=128))
    whh0_sb = loopers.tile([128, KH, NG], dt.bfloat16, name="whh0_sb")
    nc.sync.dma_start(whh0_sb[:], di["whh0"].ap().rearrange("(k p) g -> p k g", p=128))
    wih1_sb = loopers.tile([128, KH, NG], dt.bfloat16, name="wih1_sb")
    nc.sync.dma_start(wih1_sb[:], di["wih1"].ap().rearrange("(k p) g -> p k g", p=128))
    whh1_sb = loopers.tile([128, KH, NG], dt.bfloat16, name="whh1_sb")
    nc.sync.dma_start(whh1_sb[:], di["whh1"].ap().rearrange("(k p) g -> p k g", p=128))
    vvec_sb = loopers.tile([128, KH, 1], dt.bfloat16, name="vvec_sb")
    nc.sync.dma_start(vvec_sb[:], di["vvec"].ap().rearrange("(k p) o -> p k o", p=128))
    attn_b_sb = loopers.tile([1, H], dt.bfloat16, name="attn_b_sb")
    nc.sync.dma_start(attn_b_sb[:], di["attn_bias"].ap())
    bias_g0_sb = loopers.tile([1, NG], dt.bfloat16, name="bias_g0_sb")
    nc.sync.dma_start(bias_g0_sb[:], di["bias_g0"].ap())
    bias_g1_sb = loopers.tile([1, NG], dt.bfloat16, name="bias_g1_sb")
    nc.sync.dma_start(bias_g1_sb[:], di["bias_g1"].ap())
    sel_sb = loopers.tile([B, BPC], dt.bfloat16, name="sel_sb")
    nc.sync.dma_start(sel_sb[:], di["sel"].ap())
    h0t_pp = [
        loopers.tile([128, KH, B], dt.bfloat16, name=f"h0t_pp{i}") for i in range(2)
    ]
    nc.sync.dma_start(
        h0t_pp[0][:], di["h0t_init"].ap().rearrange("(k p) b -> p k b", p=128)
    )
    h1t_pp = [
        loopers.tile([128, KH, B], dt.bfloat16, name=f"h1t_pp{i}") for i in range(2)
    ]
    nc.sync.dma_start(
        h1t_pp[0][:], di["h1t_init"].ap().rearrange("(k p) b -> p k b", p=128)
    )
    c_l0 = loopers.tile([B, GS], dt.float32, name="c_l0")
    nc.sync.dma_start(c_l0[:], di["c0_l0"].ap())
    c_l1 = loopers.tile([B, GS], dt.float32, name="c_l1")
    nc.sync.dma_start(c_l1[:], di["c0_l1"].ap())

    emb_t = loopers.tile([128, KE, NSAMP], dt.bfloat16, name="emb_t")
    nc.gpsimd.memset(emb_t[:], 0.0)
    encw2 = loopers.tile([128, BPC, KH, T], dt.bfloat16, name="encw2")

    # ---- phase 1a: embedding gather + transpose to feature-major ----
    with tc.tile_pool(name="p1e", bufs=3) as p1e, \
         tc.tile_pool(name="p1eps", bufs=3, space="PSUM") as p1eps:
        for (m0, mr) in MTILES:
            idx = p1e.tile([128, 1], dt.int32, tag="idx")
            nc.sync.dma_start(idx[:mr, :], di["qidx"].ap()[m0:m0 + mr, :])
            gath = p1e.tile([128, E], dt.float32, tag="gath")
            nc.gpsimd.indirect_dma_start(
                out=gath[:mr, :],
                out_offset=None,
                in_=di["emb_tab"].ap(),
                in_offset=bass.IndirectOffsetOnAxis(ap=idx[:mr, 0:1], axis=0),
            )
            for k in range(KE):
                cw = min(128, E - k * 128)
                ps = p1eps.tile([128, 128], dt.float32, tag="ps")
                nc.tensor.transpose(
                    ps[:cw, :mr], gath[:mr, k * 128:k * 128 + cw], id_f32[:mr, :mr]
                )
                nc.vector.tensor_copy(emb_t[:cw, k, m0:m0 + mr], ps[:cw, :mr])

    # ---- phase 1b: encW2[b] = (enc_out[b] @ W2.T).T  (feature-major) ----
    with tc.tile_pool(name="p1w", bufs=1) as p1w, \
         tc.tile_pool(name="p1s", bufs=3) as p1s, \
         tc.tile_pool(name="p1ps", bufs=8, space="PSUM") as p1ps:
        w2t_sb = p1w.tile([128, KH, H], dt.bfloat16, name="w2t_sb")
        nc.sync.dma_start(
            w2t_sb[:], di["w2t"].ap().rearrange("(k p) h -> p k h", p=128)
        )
        for b in range(BPC):
            pss = [p1ps.tile([128, T], dt.float32, tag="p1p", name=f"p1p{_m}") for _m in range(KH)]
            for k in range(KH):
                rhs = p1s.tile([128, T], dt.bfloat16, tag="rhs")
                nc.sync.dma_start(
                    rhs[:], di["enc_tr"].ap()[b, k * 128:(k + 1) * 128, :]
                )
                for m in range(KH):
                    nc.tensor.matmul(
                        pss[m][:],
                        w2t_sb[:, k, m * 128:(m + 1) * 128],
                        rhs[:],
                        start=(k == 0),
                        stop=(k == KH - 1),
                    )
            for m in range(KH):
                nc.vector.tensor_copy(encw2[:, b, m, :], pss[m][:])

    # ---------------- phase 2: the recurrent loop ----------------
    sbw_cm = tc.tile_pool(name="sbw", bufs=2)
    sbw = sbw_cm.__enter__()
    psA_cm = tc.tile_pool(name="psA", bufs=2, space="PSUM")
    psA = psA_cm.__enter__()
    psB_cm = tc.tile_pool(name="psB", bufs=2, space="PSUM")
    psB = psB_cm.__enter__()
    psC_cm = tc.tile_pool(name="psC", bufs=2, space="PSUM")
    psC = psC_cm.__enter__()

    def transpose_to(dst_ap, src_ap, rows, cols, ident):
        """dst[cols,rows] (sbuf) = src[rows,cols].T via PE + copy."""
        ps = psC.tile([128, 128], src_ap.dtype, tag="ps_tr")
        nc.tensor.transpose(ps[:cols, :rows], src_ap, ident[:rows, :rows])
        nc.vector.tensor_copy(dst_ap, ps[:cols, :rows])

    def lstm_gates(gps, c_old, c_new, tag):
        """gate order i|f|g|o (each GS wide). returns h (B, GS) bf16 tile."""
        sif_r = sbw.tile([B, 2 * GS], dt.float32, tag=f"sifr{tag}")
        nc.scalar.activation(sif_r[:], gps[:, 0:2 * GS], AFT.Tanh, scale=0.5)
        sif = sbw.tile([B, 2 * GS], dt.float32, tag=f"sif{tag}")
        nc.vector.tensor_scalar(sif[:], sif_r[:], 0.5, 0.5, ALU.mult, ALU.add)
        tg = sbw.tile([B, GS], dt.float32, tag=f"tg{tag}")
        nc.scalar.activation(tg[:], gps[:, 2 * GS:3 * GS], AFT.Tanh)
        so_r = sbw.tile([B, GS], dt.float32, tag=f"sor{tag}")
        nc.scalar.activation(so_r[:], gps[:, 3 * GS:4 * GS], AFT.Tanh, scale=0.5)
        so = sbw.tile([B, GS], dt.float32, tag=f"so{tag}")
        nc.vector.tensor_scalar(so[:], so_r[:], 0.5, 0.5, ALU.mult, ALU.add)
        t_fc = sbw.tile([B, GS], dt.float32, tag=f"tfc{tag}")
        nc.vector.tensor_tensor(t_fc[:], sif[:, GS:2 * GS], c_old[:], op=ALU.mult)
        t_ig = sbw.tile([B, GS], dt.float32, tag=f"tig{tag}")
        nc.vector.tensor_tensor(t_ig[:], sif[:, 0:GS], tg[:], op=ALU.mult)
        nc.vector.tensor_tensor(c_new[:], t_fc[:], t_ig[:], op=ALU.add)
        tc2 = sbw.tile([B, GS], dt.float32, tag=f"tc2{tag}")
        nc.scalar.activation(tc2[:], c_new[:], AFT.Tanh)
        h = sbw.tile([B, GS], dt.bfloat16, tag=f"h{tag}")
        nc.vector.tensor_tensor(h[:], so[:], tc2[:], op=ALU.mult)
        return h

    for s in range(S_EFF):
        h1t_prev = h1t_pp[s % 2][:]
        h0t_prev = h0t_pp[s % 2]

        # --- hidW for all batches: (B, H) = h1.T(T) @ W1.T + attn_b ---
        ps_hw = psA.tile([B, H], dt.float32, tag="psA")
        for half in range(2):
            hs = slice(half * 512, (half + 1) * 512)
            nc.tensor.matmul(
                ps_hw[:, hs], ones_bf[:, :B], attn_b_sb[:, hs], start=True, stop=False
            )
            for k in range(KH):
                nc.tensor.matmul(
                    ps_hw[:, hs],
                    h1t_prev[:, k, :],
                    w1t_sb[:, k, hs],
                    start=False,
                    stop=(k == KH - 1),
                )
        hw_all = sbw.tile([B, H], dt.bfloat16, tag="hw_all", bufs=1)
        nc.vector.tensor_copy(hw_all[:], ps_hw[:])
        # --- select this core's 4 batches: hidW_own = sel.T @ hidW_all ---
        ps_own = psA.tile([BPC, H], dt.float32, tag="psA")
        for half in range(2):
            hs = slice(half * 512, (half + 1) * 512)
            nc.tensor.matmul(
                ps_own[:, hs], sel_sb[:], hw_all[:, hs], start=True, stop=True
            )
        hw_own = sbw.tile([BPC, H], dt.float32, tag="hw_own", bufs=1)
        nc.vector.tensor_copy(hw_own[:], ps_own[:])
        hwt = sbw.tile([128, KH, BPC], dt.float32, tag="hwt")
        for k in range(KH):
            transpose_to(
                hwt[:, k, :], hw_own[:, k * 128:(k + 1) * 128], BPC, 128, id_f32
            )

        # --- attention per local batch ---
        aw_n = sbw.tile([BPC, T], dt.bfloat16, tag="aw_n", bufs=1)
        for b in range(BPC):
            ps_sc = psB.tile([1, T], dt.float32, tag="psB")
            for k in range(KH):
                en = sbw.tile([128, T], dt.bfloat16, tag="energy")
                nc.scalar.activation(
                    en[:], encw2[:, b, k, :], AFT.Tanh, bias=hwt[:, k, b:b + 1]
                )
                nc.tensor.matmul(
                    ps_sc[:], vvec_sb[:, k, :], en[:],
                    start=(k == 0), stop=(k == KH - 1),
                )
            awr = sbw.tile([1, T], dt.float32, tag=f"awr{b}", name=f"awr{b}", bufs=1)
            den = sbw.tile([1, 1], dt.float32, tag=f"den{b}", name=f"den{b}", bufs=1)
            nc.scalar.activation(
                awr[:], ps_sc[:], AFT.Exp, accum_out=den[:, 0:1]
            )
            rec = sbw.tile([1, 1], dt.float32, tag=f"rec{b}", name=f"rec{b}", bufs=1)
            nc.vector.reciprocal(rec[:], den[:])
            awn = sbw.tile([1, T], dt.bfloat16, tag=f"awn{b}", name=f"awn{b}", bufs=1)
            nc.vector.tensor_scalar(awn[:], awr[:], rec[:, 0:1], None, ALU.mult)
            nc.sync.dma_start(aw_n[b:b + 1, :], awn[:])
        awt = sbw.tile([128, KT, BPC], dt.bfloat16, tag="awt")
        for t in range(KT):
            transpose_to(
                awt[:, t, :], aw_n[:, t * 128:(t + 1) * 128], BPC, 128, id_bf
            )

        # --- context rows then transpose to feature-major ---
        ctx_rows = sbw.tile([BPC, H], dt.bfloat16, tag="ctx_rows", bufs=1)
        for b in range(BPC):
            ps_cx = psA.tile([1, H], dt.float32, tag="psA")
            for half in range(2):
                hs = slice(half * 512, (half + 1) * 512)
                for t in range(KT):
                    nc.tensor.matmul(
                        ps_cx[:, hs],
                        awt[:, t, b:b + 1],
                        enc_nat_sb[:, b, t, hs],
                        start=(t == 0),
                        stop=(t == KT - 1),
                    )
            cxr = sbw.tile([1, H], dt.bfloat16, tag=f"cxr{b}", name=f"cxr{b}", bufs=1)
            nc.any.tensor_copy(cxr[:], ps_cx[:])
            nc.sync.dma_start(ctx_rows[b:b + 1, :], cxr[:])
        ctxt = sbw.tile([128, KH, BPC], dt.bfloat16, tag="ctxt")
        for k in range(KH):
            transpose_to(
                ctxt[:, k, :], ctx_rows[:, k * 128:(k + 1) * 128], BPC, 128, id_bf
            )
        bx_in = dram.tile([H, BPC], dt.bfloat16, tag="bx_in", bufs=3)
        nc.sync.dma_start(
            bx_in[:].rearrange("(k p) b -> p k b", p=128), ctxt[:]
        )
        bx_out = dram.tile(
            [NCORES * H, BPC], dt.bfloat16, tag="bx_out", bufs=3, addr_space="Shared"
        )
        nc.gpsimd.collective_compute(
            "AllGather", ALU.bypass, replica_groups=RG,
            ins=[bx_in[:].opt()], outs=[bx_out[:].opt()],
        )
        xt_ctx = sbw.tile([128, KH, NCORES, BPC], dt.bfloat16, tag="xt_ctx")
        for _k in range(KH):
            nc.sync.dma_start(
                xt_ctx[:, _k, :, :],
                bx_out[:].rearrange("(c k p) b -> k p c b", p=128, c=NCORES)[_k],
            )

        # --- LSTM layer 0 (tensor-parallel gates) ---
        ps_g0 = psB.tile([B, NG], dt.float32, tag="psB")
        nc.tensor.matmul(ps_g0[:], ones_bf[:, :B], bias_g0_sb[:], start=True, stop=False)
        for k in range(KE):
            nc.tensor.matmul(
                ps_g0[:], emb_t[:, k, s * B:(s + 1) * B], wih0e_sb[:, k, :],
                start=False, stop=False,
            )
        for k in range(KH):
            nc.tensor.matmul(
                ps_g0[:], xt_ctx[:, k, :, :], wih0c_sb[:, k, :],
                start=False, stop=False,
            )
        for k in range(KH):
            nc.tensor.matmul(
                ps_g0[:], h0t_prev[:, k, :], whh0_sb[:, k, :],
                start=False, stop=(k == KH - 1),
            )
        c_l0_new = sbw.tile([B, GS], dt.float32, tag="c_l0n", bufs=2)
        h0n = lstm_gates(ps_g0, c_l0, c_l0_new, "l0")
        c_l0 = c_l0_new
        h0ts = sbw.tile([128, B], dt.bfloat16, tag="h0ts")
        transpose_to(h0ts[:], h0n[:], B, 128, id_bf)
        bh0_in = dram.tile([GS, B], dt.bfloat16, tag="bh0_in", bufs=3)
        nc.sync.dma_start(bh0_in[:], h0ts[:])
        bh0_out = dram.tile(
            [NCORES * GS, B], dt.bfloat16, tag="bh0_out", bufs=3, addr_space="Shared"
        )
        nc.gpsimd.collective_compute(
            "AllGather", ALU.bypass, replica_groups=RG,
            ins=[bh0_in[:].opt()], outs=[bh0_out[:].opt()],
        )
        h0t_new = h0t_pp[(s + 1) % 2]
        nc.sync.dma_start(
            h0t_new[:], bh0_out[:].rearrange("(k p) b -> p k b", p=128)
        )

        # --- LSTM layer 1 ---
        ps_g1 = psB.tile([B, NG], dt.float32, tag="psB")
        nc.tensor.matmul(ps_g1[:], ones_bf[:, :B], bias_g1_sb[:], start=True, stop=False)
        for k in range(KH):
            nc.tensor.matmul(
                ps_g1[:], h0t_new[:, k, :], wih1_sb[:, k, :],
                start=False, stop=False,
            )
        for k in range(KH):
            nc.tensor.matmul(
                ps_g1[:], h1t_prev[:, k, :], whh1_sb[:, k, :],
                start=False, stop=(k == KH - 1),
            )
        c_l1_new = sbw.tile([B, GS], dt.float32, tag="c_l1n", bufs=2)
        h1n = lstm_gates(ps_g1, c_l1, c_l1_new, "l1")
        c_l1 = c_l1_new
        h1ts = sbw.tile([128, B], dt.bfloat16, tag="h1ts")
        transpose_to(h1ts[:], h1n[:], B, 128, id_bf)
        bh1_in = dram.tile([GS, B], dt.bfloat16, tag="bh1_in", bufs=3)
        nc.sync.dma_start(bh1_in[:], h1ts[:])
        bh1_out = hall_steps[s][:]
        nc.gpsimd.collective_compute(
            "AllGather", ALU.bypass, replica_groups=RG,
            ins=[bh1_in[:].opt()], outs=[bh1_out.opt()],
        )
        nc.sync.dma_start(
            h1t_pp[(s + 1) % 2][:],
            bh1_out.rearrange("(k p) b -> p k b", p=128),
        )

    # close loop pools
    psC_cm.__exit__(None, None, None)
    psB_cm.__exit__(None, None, None)
    psA_cm.__exit__(None, None, None)
    sbw_cm.__exit__(None, None, None)
    loopers_cm.__exit__(None, None, None)

    # ---------------- phase 4: vocab projection + exp-sums ----------------
    logits_d = dram.tile([NSAMP, VPC], dt.float32, name="logits_d")
    with tc.tile_pool(name="p4", bufs=3) as p4, \
         tc.tile_pool(name="p4c", bufs=1) as p4c, \
         tc.tile_pool(name="p4ps", bufs=4, space="PSUM") as p4ps:
        genb_sb = p4c.tile([1, VPC], dt.bfloat16, name="genb_sb")
        nc.sync.dma_start(genb_sb[:], di["genb_v"].ap())
        for (m0, mr) in MTILES:
            s0 = m0 // B
            ns = mr // B
            hh = p4.tile([128, KH, 4, B], dt.bfloat16, tag="hh")
            for k in range(KH):
                for sl in range(ns):
                    nc.sync.dma_start(
                        hh[:, k, sl, :],
                        hall_steps[s0 + sl][:].rearrange(
                            "(k p) b -> k p b", p=128
                        )[k],
                    )
            sparts = p4.tile([128, len(VCH)], dt.float32, tag="sparts")
            for n, cw in enumerate(VCH):
                gw = p4.tile([128, KH, 512], dt.bfloat16, tag="gw")
                nc.sync.dma_start(
                    gw[:, :, :cw],
                    di["genw_t"].ap()[:, VOFF[n]:VOFF[n] + cw].rearrange(
                        "(k p) v -> p k v", p=128
                    ),
                )
                ps = p4ps.tile([128, 512], dt.float32, tag="p4p")
                nc.tensor.matmul(
                    ps[:mr, :cw], ones_bf[:, :mr],
                    genb_sb[:, VOFF[n]:VOFF[n] + cw], start=True, stop=False,
                )
                for k in range(KH):
                    nc.tensor.matmul(
                        ps[:mr, :cw],
                        hh[:, k, :ns, :],
                        gw[:, k, :cw],
                        start=False, stop=(k == KH - 1),
                    )
                scr = p4.tile([128, 512], dt.bfloat16, tag="scr")
                nc.scalar.activation(
                    scr[:mr, :cw], ps[:mr, :cw], AFT.Exp,
                    accum_out=sparts[:mr, n:n + 1],
                )
                lg = p4.tile([128, 512], dt.float32, tag="lg")
                nc.vector.tensor_copy(lg[:mr, :cw], ps[:mr, :cw])
                nc.sync.dma_start(
                    logits_d[:][m0:m0 + mr, VOFF[n]:VOFF[n] + cw], lg[:mr, :cw]
                )
            m = m0 // 128
            nc.vector.tensor_reduce(
                sume_all[:mr, m:m + 1], sparts[:mr, :],
                axis=mybir.AxisListType.X, op=ALU.add,
            )

    # ---------------- phase 5: global logsumexp + subtract ----------------
    nm = len(MTILES)
    blse_in = dram.tile([128, nm], dt.float32, name="blse_in")
    blse_out = dram.tile([128, nm], dt.float32, name="blse_out", addr_space="Shared")
    nc.sync.dma_start(blse_in[:], sume_all[:])
    nc.gpsimd.collective_compute(
        "AllReduce", ALU.add, replica_groups=RG,
        ins=[blse_in[:].opt()], outs=[blse_out[:].opt()],
    )
    with tc.tile_pool(name="p5", bufs=2) as p5:
        sume_g = p5.tile([128, nm], dt.float32, name="sume_g", bufs=1)
        nc.sync.dma_start(sume_g[:], blse_out[:])
        lse = p5.tile([128, nm], dt.float32, name="lse", bufs=1)
        nc.scalar.activation(lse[:], sume_g[:], AFT.Ln)
        for (m0, mr) in MTILES:
            m = m0 // 128
            lgi = p5.tile([128, VPC], dt.float32, tag="lgi")
            nc.sync.dma_start(lgi[:mr, :], logits_d[:][m0:m0 + mr, :])
            lpo = p5.tile([128, VPC], dt.float32, tag="lpo")
            nc.vector.tensor_scalar(
                lpo[:mr, :], lgi[:mr, :], lse[:mr, m:m + 1], None, ALU.subtract
            )
            nc.sync.dma_start(logp.ap()[m0:m0 + mr, :], lpo[:mr, :])

    dram_cm.__exit__(None, None, None)
    glob_cm.__exit__(None, None, None)


def _prep_inputs(inputs):
    """Host-side sharding/layout prep. Returns list of per-core input dicts."""
    f32 = np.float32
    enc_out = np.asarray(inputs["enc_out"], f32)
    enc_h = np.asarray(inputs["enc_h"], f32)
    enc_c = np.asarray(inputs["enc_c"], f32)
    emb = np.asarray(inputs["embedding"], f32)
    attn_W = np.asarray(inputs["attn_W"], f32)
    attn_b = np.asarray(inputs["attn_b"], f32)
    vv = np.asarray(inputs["v"], f32)
    Wih0 = np.asarray(inputs["Wih0"], f32)
    Whh0 = np.asarray(inputs["Whh0"], f32)
    bih0 = np.asarray(inputs["bih0"], f32)
    bhh0 = np.asarray(inputs["bhh0"], f32)
    Wih1 = np.asarray(inputs["Wih1"], f32)
    Whh1 = np.asarray(inputs["Whh1"], f32)
    bih1 = np.asarray(inputs["bih1"], f32)
    bhh1 = np.asarray(inputs["bhh1"], f32)
    genW = np.asarray(inputs["genW"], f32)
    genb = np.asarray(inputs["genb"], f32)
    q = np.asarray(inputs["question"]).astype(np.int64)

    W1 = attn_W[:, :H]
    W2 = attn_W[:, H:]
    h0 = np.concatenate([enc_h[0], enc_h[1]], 1)  # (B, H) layer 0
    h1 = np.concatenate([enc_h[2], enc_h[3]], 1)  # layer 1
    c0 = np.concatenate([enc_c[0], enc_c[1]], 1)
    c1 = np.concatenate([enc_c[2], enc_c[3]], 1)
    qflat = q.T.reshape(NSAMP, 1).astype(np.int32)  # (s,b) order

    def bf(x):
        return np.ascontiguousarray(x).astype(BF)

    shared = {
        "w2t": bf(W2.T),
        "w1t": bf(W1.T),
        "attn_bias": bf(attn_b.reshape(1, H)),
        "vvec": bf(vv.reshape(H, 1)),
        "emb_tab": np.ascontiguousarray(emb),
        "qidx": qflat,
        "h0t_init": bf(h0.T),
        "h1t_init": bf(h1.T),
    }
    maps = []
    for c in range(NCORES):
        bs = slice(c * BPC, (c + 1) * BPC)
        rows = np.concatenate(
            [np.arange(g * H + c * GS, g * H + (c + 1) * GS) for g in range(4)]
        )
        wih0_s = Wih0[rows]  # (NG, E+H)
        wih0e = np.zeros((EP, NG), f32)
        wih0e[:E] = wih0_s[:, :E].T
        sel = np.zeros((B, BPC), f32)
        for j in range(BPC):
            sel[c * BPC + j, j] = 1.0
        vrows = slice(c * VPC, (c + 1) * VPC)
        m = dict(shared)
        m.update({
            "enc_nat": bf(enc_out[bs]),
            "enc_tr": bf(enc_out[bs].transpose(0, 2, 1)),
            "wih0e": bf(wih0e),
            "wih0c": bf(wih0_s[:, E:].T),
            "whh0": bf(Whh0[rows].T),
            "wih1": bf(Wih1[rows].T),
            "whh1": bf(Whh1[rows].T),
            "bias_g0": bf((bih0 + bhh0)[rows].reshape(1, NG)),
            "bias_g1": bf((bih1 + bhh1)[rows].reshape(1, NG)),
            "sel": bf(sel),
            "c0_l0": np.ascontiguousarray(c0[:, c * GS:(c + 1) * GS]),
            "c0_l1": np.ascontiguousarray(c1[:, c * GS:(c + 1) * GS]),
            "genw_t": bf(genW[vrows].T),
            "genb_v": bf(genb[vrows].reshape(1, VPC)),
        })
        maps.append(m)
    return maps


_CACHED = {}


def _get_compiled():
    if "nc" not in _CACHED:
        nc = bacc.Bacc(
            "TRN2", target_bir_lowering=False, debug=False, num_devices=NCORES
        )
        build(nc)
        nc.compile()
        _CACHED["nc"] = nc
    return _CACHED["nc"]


def run_cores(in_maps, **kw):
    nc = _get_compiled()
    return bass_utils.run_bass_kernel_spmd(nc, in_maps, list(range(NCORES)), **kw)


def kernel(**inputs):
    in_maps = _prep_inputs(inputs)
    res = run_cores(in_maps)
    parts = [res.results[c]["logp"] for c in range(NCORES)]
    full = np.concatenate(parts, axis=1)  # (NSAMP, V)
    out = full.reshape(S, B, V).transpose(1, 0, 2)
    return np.ascontiguousarray(out.astype(np.float32))


# revision 19
# speedup vs baseline: 2111.5221x; 2111.5221x over previous
"""Trainium2 Bass kernel for the attention-LSTM decoder (nn_Decoder).

Strategy (8 NeuronCores):
  - Attention batch-sharded: each core owns B/8 = 4 batches (enc_out slice,
    encW2 precompute, tanh energies, softmax, context).
  - LSTM tensor-parallel over the hidden dim: each core owns a 128-wide
    hidden slice -> 512 gate rows per layer; h slices are AllGathered
    (feature-major) each step.
  - Vocab projection tensor-parallel over V: deferred to one big matmul over
    all (step, batch) samples at the end; global logsumexp via one AllReduce.
Dtypes: bf16 storage for big operands, fp32 PSUM accumulation everywhere,
fp32 cell state, fp32r for the embedding table.
"""
import os
import sys

sys.path.insert(0, "/opt/trn_rl_repo")

import numpy as np
import ml_dtypes

import concourse.bass as bass
import concourse.bacc as bacc
import concourse.mybir as mybir
import concourse.tile as tile
from concourse import bass_utils
from concourse.masks import make_identity

BF = ml_dtypes.bfloat16
dt = mybir.dt
AFT = mybir.ActivationFunctionType
ALU = mybir.AluOpType

B, T, H, E, V, S = 32, 512, 1024, 300, 32000, 50
NCORES = 8
BPC = B // NCORES      # 4 batches per core
GS = H // NCORES       # 128-wide hidden slice per core
NG = 4 * GS            # 512 gate rows per core
VPC = V // NCORES      # 4000 vocab rows per core
EP = 384               # padded embedding feature dim (3 k-tiles)
KE = EP // 128         # 3
KH = H // 128          # 8
KT = T // 128          # 4
NSAMP = S * B          # 1600
S_EFF = int(os.environ.get("DECODER_STEPS", str(S)))
SIM1 = os.environ.get("DECODER_SIM", "0") == "1"
ABL = set(os.environ.get("DECODER_ABL", "").split(","))
RG = [list(range(NCORES))]
SHARED = "Local" if SIM1 else "Shared"


def _allgather(nc, in_ap, out_ap):
    if SIM1:
        rows = in_ap.shape[0]
        nblk = 1 if "noag" in ABL else NCORES
        for c in range(nblk):
            nc.sync.dma_start(out_ap[c * rows:(c + 1) * rows, :], in_ap)
    else:
        nc.gpsimd.collective_compute(
            "AllGather", mybir.AluOpType.bypass, replica_groups=RG,
            ins=[in_ap.opt()], outs=[out_ap.opt()],
        )


def _allreduce(nc, in_ap, out_ap):
    if SIM1:
        nc.sync.dma_start(out_ap, in_ap)
    else:
        nc.gpsimd.collective_compute(
            "AllReduce", mybir.AluOpType.add, replica_groups=RG,
            ins=[in_ap.opt()], outs=[out_ap.opt()],
        )

# phase-4 vocab chunking
VCH = [512] * 7 + [416]
VOFF = [sum(VCH[:i]) for i in range(len(VCH))]
MTILES = [(m * 128, min(128, NSAMP - m * 128)) for m in range((NSAMP + 127) // 128)]


def build(nc):
    di = {}

    def inp(name, shape, dtype):
        di[name] = nc.dram_tensor(name, list(shape), dtype, kind="ExternalInput")
        return di[name]

    enc_nat = inp("enc_nat", (BPC, T, H), dt.bfloat16)
    enc_tr = inp("enc_tr", (BPC, H, T), dt.bfloat16)
    w2t = inp("w2t", (H, H), dt.bfloat16)
    w1t = inp("w1t", (H, H), dt.bfloat16)
    attn_bias = inp("attn_bias", (1, H), dt.bfloat16)
    vvec = inp("vvec", (H, 1), dt.bfloat16)
    emb_tab = inp("emb_tab", (V, E), dt.float32)
    qidx = inp("qidx", (NSAMP, 1), dt.int32)
    wih0e = inp("wih0e", (EP, NG), dt.bfloat16)
    wih0c = inp("wih0c", (H, NG), dt.bfloat16)
    whh0 = inp("whh0", (H, NG), dt.bfloat16)
    wih1 = inp("wih1", (H, NG), dt.bfloat16)
    whh1 = inp("whh1", (H, NG), dt.bfloat16)
    bias_g0 = inp("bias_g0", (1, NG), dt.bfloat16)
    bias_g1 = inp("bias_g1", (1, NG), dt.bfloat16)
    sel = inp("sel", (B, BPC), dt.bfloat16)
    h0t_init = inp("h0t_init", (H, B), dt.bfloat16)
    h1t_init = inp("h1t_init", (H, B), dt.bfloat16)
    c0_l0 = inp("c0_l0", (B, GS), dt.float32)
    c0_l1 = inp("c0_l1", (B, GS), dt.float32)
    genw_t = inp("genw_t", (H, VPC), dt.bfloat16)
    genb_v = inp("genb_v", (1, VPC), dt.bfloat16)
    logp = nc.dram_tensor("logp", [NSAMP, VPC], dt.float32, kind="ExternalOutput")

    with tile.TileContext(nc) as tc:
        _body(nc, tc, di, logp)
    return di


def _body(nc, tc, di, logp):
    glob_cm = tc.tile_pool(name="glob", bufs=1)
    glob = glob_cm.__enter__()
    dram_cm = tc.tile_pool(name="dram", bufs=1, space="DRAM")
    dram = dram_cm.__enter__()

    # ---- global constants ----
    id_bf = glob.tile([128, 128], dt.bfloat16, name="id_bf")
    id_f32 = glob.tile([128, 128], dt.float32, name="id_f32")
    make_identity(nc, id_bf[:])
    make_identity(nc, id_f32[:])
    ones_bf = glob.tile([1, 128], dt.bfloat16, name="ones_bf")
    nc.gpsimd.memset(ones_bf[:], 1.0)

    # h1T history lives in shared DRAM: tile s = h1T after step s
    hall_steps = [
        dram.tile([NCORES * GS, B], dt.bfloat16, name=f"hall{s}",
                  addr_space=SHARED)
        for s in range(S)
    ]
    sume_all = glob.tile([128, len(MTILES)], dt.float32, name="sume_all")

    # ---------------- phase 0/1: loop-scoped persistent tensors ----------------
    loopers_cm = tc.tile_pool(name="loopers", bufs=1)
    loopers = loopers_cm.__enter__()

    enc_nat_sb = loopers.tile([128, BPC, KT, H], dt.bfloat16, name="enc_nat_sb")
    for _b in range(BPC):
        nc.sync.dma_start(
            enc_nat_sb[:, _b, :, :],
            di["enc_nat"].ap()[_b].rearrange("(k p) h -> p k h", p=128),
        )
    w1t_sb = loopers.tile([128, KH, H], dt.bfloat16, name="w1t_sb")
    nc.sync.dma_start(w1t_sb[:], di["w1t"].ap().rearrange("(k p) h -> p k h", p=128))
    wih0e_sb = loopers.tile([128, KE, NG], dt.bfloat16, name="wih0e_sb")
    nc.sync.dma_start(wih0e_sb[:], di["wih0e"].ap().rearrange("(k p) g -> p k g", p=128))
    wih0c_sb = loopers.tile([128, KH, NG], dt.bfloat16, name="wih0c_sb")
    nc.sync.dma_start(wih0c_sb[:], di["wih0c"].ap().rearrange("(k p) g -> p k g", p=128))
    whh0_sb = loopers.tile([128, KH, NG], dt.bfloat16, name="whh0_sb")
    nc.sync.dma_start(whh0_sb[:], di["whh0"].ap().rearrange("(k p) g -> p k g", p=128))
    wih1_sb = loopers.tile([128, KH, NG], dt.bfloat16, name="wih1_sb")
    nc.sync.dma_start(wih1_sb[:], di["wih1"].ap().rearrange("(k p) g -> p k g", p=128))
    whh1_sb = loopers.tile([128, KH, NG], dt.bfloat16, name="whh1_sb")
    nc.sync.dma_start(whh1_sb[:], di["whh1"].ap().rearrange("(k p) g -> p k g", p=128))
    vvec_sb = loopers.tile([128, KH, 1], dt.bfloat16, name="vvec_sb")
    nc.sync.dma_start(vvec_sb[:], di["vvec"].ap().rearrange("(k p) o -> p k o", p=128))
    attn_b_sb = loopers.tile([1, H], dt.bfloat16, name="attn_b_sb")
    nc.sync.dma_start(attn_b_sb[:], di["attn_bias"].ap())
    bias_g0_sb = loopers.tile([1, NG], dt.bfloat16, name="bias_g0_sb")
    nc.sync.dma_start(bias_g0_sb[:], di["bias_g0"].ap())
    bias_g1_sb = loopers.tile([1, NG], dt.bfloat16, name="bias_g1_sb")
    nc.sync.dma_start(bias_g1_sb[:], di["bias_g1"].ap())
    sel_sb = loopers.tile([B, BPC], dt.bfloat16, name="sel_sb")
    nc.sync.dma_start(sel_sb[:], di["sel"].ap())
    h0t_pp = [
        loopers.tile([128, KH, B], dt.bfloat16, name=f"h0t_pp{i}") for i in range(2)
    ]
    nc.sync.dma_start(
        h0t_pp[0][:], di["h0t_init"].ap().rearrange("(k p) b -> p k b", p=128)
    )
    h1t_pp = [
        loopers.tile([128, KH, B], dt.bfloat16, name=f"h1t_pp{i}") for i in range(2)
    ]
    nc.sync.dma_start(
        h1t_pp[0][:], di["h1t_init"].ap().rearrange("(k p) b -> p k b", p=128)
    )
    c_l0 = loopers.tile([B, GS], dt.float32, name="c_l0")
    nc.sync.dma_start(c_l0[:], di["c0_l0"].ap())
    c_l1 = loopers.tile([B, GS], dt.float32, name="c_l1")
    nc.sync.dma_start(c_l1[:], di["c0_l1"].ap())

    emb_t = loopers.tile([128, KE, NSAMP], dt.bfloat16, name="emb_t")
    nc.gpsimd.memset(emb_t[:], 0.0)
    encw2 = loopers.tile([128, BPC, KH, T], dt.bfloat16, name="encw2")

    # ---- phase 1a: embedding gather + transpose to feature-major ----
    with tc.tile_pool(name="p1e", bufs=3) as p1e, \
         tc.tile_pool(name="p1eps", bufs=3, space="PSUM") as p1eps:
        for (m0, mr) in MTILES:
            idx = p1e.tile([128, 1], dt.int32, tag="idx")
            nc.sync.dma_start(idx[:mr, :], di["qidx"].ap()[m0:m0 + mr, :])
            gath = p1e.tile([128, E], dt.float32, tag="gath")
            nc.gpsimd.indirect_dma_start(
                out=gath[:mr, :],
                out_offset=None,
                in_=di["emb_tab"].ap(),
                in_offset=bass.IndirectOffsetOnAxis(ap=idx[:mr, 0:1], axis=0),
            )
            for k in range(KE):
                cw = min(128, E - k * 128)
                ps = p1eps.tile([128, 128], dt.float32, tag="ps")
                nc.tensor.transpose(
                    ps[:cw, :mr], gath[:mr, k * 128:k * 128 + cw], id_f32[:mr, :mr]
                )
                nc.vector.tensor_copy(emb_t[:cw, k, m0:m0 + mr], ps[:cw, :mr])

    # ---- phase 1b: encW2[b] = (enc_out[b] @ W2.T).T  (feature-major) ----
    with tc.tile_pool(name="p1w", bufs=1) as p1w, \
         tc.tile_pool(name="p1s", bufs=3) as p1s, \
         tc.tile_pool(name="p1ps", bufs=8, space="PSUM") as p1ps:
        w2t_sb = p1w.tile([128, KH, H], dt.bfloat16, name="w2t_sb")
        nc.sync.dma_start(
            w2t_sb[:], di["w2t"].ap().rearrange("(k p) h -> p k h", p=128)
        )
        for b in range(BPC):
            pss = [p1ps.tile([128, T], dt.float32, tag="p1p", name=f"p1p{_m}") for _m in range(KH)]
            for k in range(KH):
                rhs = p1s.tile([128, T], dt.bfloat16, tag="rhs")
                nc.sync.dma_start(
                    rhs[:], di["enc_tr"].ap()[b, k * 128:(k + 1) * 128, :]
                )
                for m in range(KH):
                    nc.tensor.matmul(
                        pss[m][:],
                        w2t_sb[:, k, m * 128:(m + 1) * 128],
                        rhs[:],
                        start=(k == 0),
                        stop=(k == KH - 1),
                    )
            for m in range(KH):
                nc.vector.tensor_copy(encw2[:, b, m, :], pss[m][:])

    # ---------------- phase 2: the recurrent loop ----------------
    sbw_cm = tc.tile_pool(name="sbw", bufs=2)
    sbw = sbw_cm.__enter__()
    psA_cm = tc.tile_pool(name="psA", bufs=2, space="PSUM")
    psA = psA_cm.__enter__()
    psB_cm = tc.tile_pool(name="psB", bufs=2, space="PSUM")
    psB = psB_cm.__enter__()
    psC_cm = tc.tile_pool(name="psC", bufs=2, space="PSUM")
    psC = psC_cm.__enter__()

    def transpose_to(dst_ap, src_ap, rows, cols, ident):
        """dst[cols,rows] (sbuf) = src[rows,cols].T via PE + copy."""
        ps = psC.tile([128, 128], src_ap.dtype, tag="ps_tr")
        nc.tensor.transpose(ps[:cols, :rows], src_ap, ident[:rows, :rows])
        nc.vector.tensor_copy(dst_ap, ps[:cols, :rows])

    def lstm_gates(gps, c_old, c_new, tag):
        """gate order i|f|g|o (each GS wide). returns h (B, GS) bf16 tile."""
        sif_r = sbw.tile([B, 2 * GS], dt.float32, tag=f"sifr{tag}")
        nc.scalar.activation(sif_r[:], gps[:, 0:2 * GS], AFT.Tanh, scale=0.5)
        sif = sbw.tile([B, 2 * GS], dt.float32, tag=f"sif{tag}")
        nc.vector.tensor_scalar(sif[:], sif_r[:], 0.5, 0.5, ALU.mult, ALU.add)
        tg = sbw.tile([B, GS], dt.float32, tag=f"tg{tag}")
        nc.scalar.activation(tg[:], gps[:, 2 * GS:3 * GS], AFT.Tanh)
        so_r = sbw.tile([B, GS], dt.float32, tag=f"sor{tag}")
        nc.scalar.activation(so_r[:], gps[:, 3 * GS:4 * GS], AFT.Tanh, scale=0.5)
        so = sbw.tile([B, GS], dt.float32, tag=f"so{tag}")
        nc.vector.tensor_scalar(so[:], so_r[:], 0.5, 0.5, ALU.mult, ALU.add)
        t_fc = sbw.tile([B, GS], dt.float32, tag=f"tfc{tag}")
        nc.vector.tensor_tensor(t_fc[:], sif[:, GS:2 * GS], c_old[:], op=ALU.mult)
        t_ig = sbw.tile([B, GS], dt.float32, tag=f"tig{tag}")
        nc.vector.tensor_tensor(t_ig[:], sif[:, 0:GS], tg[:], op=ALU.mult)
        nc.vector.tensor_tensor(c_new[:], t_fc[:], t_ig[:], op=ALU.add)
        tc2 = sbw.tile([B, GS], dt.float32, tag=f"tc2{tag}")
        nc.scalar.activation(tc2[:], c_new[:], AFT.Tanh)
        h = sbw.tile([B, GS], dt.bfloat16, tag=f"h{tag}")
        nc.vector.tensor_tensor(h[:], so[:], tc2[:], op=ALU.mult)
        return h

    for s in range(S_EFF):
        h1t_prev = h1t_pp[s % 2][:]
        h0t_prev = h0t_pp[s % 2]

        # --- hidW for all batches: (B, H) = h1.T(T) @ W1.T + attn_b ---
        ps_hw = psA.tile([B, H], dt.float32, tag="psA")
        for half in range(2):
            hs = slice(half * 512, (half + 1) * 512)
            nc.tensor.matmul(
                ps_hw[:, hs], ones_bf[:, :B], attn_b_sb[:, hs], start=True, stop=False
            )
            for k in range(KH):
                nc.tensor.matmul(
                    ps_hw[:, hs],
                    h1t_prev[:, k, :],
                    w1t_sb[:, k, hs],
                    start=False,
                    stop=(k == KH - 1),
                )
        hw_all = sbw.tile([B, H], dt.bfloat16, tag="hw_all", bufs=1)
        nc.vector.tensor_copy(hw_all[:], ps_hw[:])
        # --- select this core's 4 batches: hidW_own = sel.T @ hidW_all ---
        ps_own = psA.tile([BPC, H], dt.float32, tag="psA")
        for half in range(2):
            hs = slice(half * 512, (half + 1) * 512)
            nc.tensor.matmul(
                ps_own[:, hs], sel_sb[:], hw_all[:, hs], start=True, stop=True
            )
        hw_own = sbw.tile([BPC, H], dt.float32, tag="hw_own", bufs=1)
        nc.vector.tensor_copy(hw_own[:], ps_own[:])
        hwt = sbw.tile([128, KH, BPC], dt.float32, tag="hwt")
        for k in range(KH):
            transpose_to(
                hwt[:, k, :], hw_own[:, k * 128:(k + 1) * 128], BPC, 128, id_f32
            )

        # --- attention per local batch ---
        awt = sbw.tile([128, KT, BPC], dt.bfloat16, tag="awt")
        for b in range(BPC):
            ps_sc = psB.tile([1, T], dt.float32, tag="psB")
            for k in range(KH):
                en = sbw.tile([128, T], dt.bfloat16, tag="energy")
                if "notanh" not in ABL:
                    nc.scalar.activation(
                        en[:], encw2[:, b, k, :], AFT.Tanh, bias=hwt[:, k, b:b + 1]
                    )
                nc.tensor.matmul(
                    ps_sc[:], vvec_sb[:, k, :], en[:],
                    start=(k == 0), stop=(k == KH - 1),
                )
            awr = sbw.tile([1, T], dt.float32, tag=f"awr{b}", name=f"awr{b}", bufs=1)
            den = sbw.tile([1, 1], dt.float32, tag=f"den{b}", name=f"den{b}", bufs=1)
            nc.scalar.activation(
                awr[:], ps_sc[:], AFT.Exp, accum_out=den[:, 0:1]
            )
            rec = sbw.tile([1, 1], dt.float32, tag=f"rec{b}", name=f"rec{b}", bufs=1)
            nc.vector.reciprocal(rec[:], den[:])
            awn = sbw.tile([1, T], dt.bfloat16, tag=f"awn{b}", name=f"awn{b}", bufs=1)
            nc.vector.tensor_scalar(awn[:], awr[:], rec[:, 0:1], None, ALU.mult)
            for t in range(KT):
                pst = psC.tile([128, 128], dt.bfloat16, tag="ps_tr")
                nc.tensor.transpose(
                    pst[:, :1], awn[:, t * 128:(t + 1) * 128], id_bf[:1, :1]
                )
                nc.vector.tensor_copy(awt[:, t, b:b + 1], pst[:, :1])

        # --- context rows then transpose to feature-major ---
        ctx_rows = sbw.tile([BPC, H], dt.bfloat16, tag="ctx_rows", bufs=1)
        cxw = sbw.tile([1, BPC * H], dt.bfloat16, tag="cxw", bufs=1)
        for b in ([] if "noctx" in ABL else range(BPC)):
            ps_cx = psA.tile([1, H], dt.float32, tag="psA")
            for half in range(2):
                hs = slice(half * 512, (half + 1) * 512)
                for t in range(KT):
                    nc.tensor.matmul(
                        ps_cx[:, hs],
                        awt[:, t, b:b + 1],
                        enc_nat_sb[:, b, t, hs],
                        start=(t == 0),
                        stop=(t == KT - 1),
                    )
            nc.any.tensor_copy(cxw[:, b * H:(b + 1) * H], ps_cx[:])
        for b in range(BPC):
            nc.sync.dma_start(
                ctx_rows[b:b + 1, :], cxw[:, b * H:(b + 1) * H]
            )
        ctxt = sbw.tile([128, KH, BPC], dt.bfloat16, tag="ctxt")
        for k in range(KH):
            transpose_to(
                ctxt[:, k, :], ctx_rows[:, k * 128:(k + 1) * 128], BPC, 128, id_bf
            )
        bx_in = dram.tile([H, BPC], dt.bfloat16, tag="bx_in", bufs=3)
        nc.sync.dma_start(
            bx_in[:].rearrange("(k p) b -> p k b", p=128), ctxt[:]
        )
        bx_out = dram.tile(
            [NCORES * H, BPC], dt.bfloat16, tag="bx_out", bufs=3, addr_space=SHARED
        )
        _allgather(nc, bx_in[:], bx_out[:])
        xt_ctx = sbw.tile([128, KH, NCORES, BPC], dt.bfloat16, tag="xt_ctx")
        for _k in range(KH):
            nc.sync.dma_start(
                xt_ctx[:, _k, :, :],
                bx_out[:].rearrange("(c k p) b -> k p c b", p=128, c=NCORES)[_k],
            )

        # --- LSTM layer 0 (tensor-parallel gates) ---
        ps_g0 = psB.tile([B, NG], dt.float32, tag="psB")
        nc.tensor.matmul(ps_g0[:], ones_bf[:, :B], bias_g0_sb[:], start=True, stop=False)
        for k in range(KE):
            nc.tensor.matmul(
                ps_g0[:], emb_t[:, k, s * B:(s + 1) * B], wih0e_sb[:, k, :],
                start=False, stop=False,
            )
        for k in range(KH):
            nc.tensor.matmul(
                ps_g0[:], h0t_prev[:, k, :], whh0_sb[:, k, :],
                start=False, stop=False,
            )
        for k in range(KH):
            nc.tensor.matmul(
                ps_g0[:], xt_ctx[:, k, :, :], wih0c_sb[:, k, :],
                start=False, stop=(k == KH - 1),
            )
        c_l0_new = sbw.tile([B, GS], dt.float32, tag="c_l0n", bufs=2)
        h0n = lstm_gates(ps_g0, c_l0, c_l0_new, "l0")
        c_l0 = c_l0_new
        h0ts = sbw.tile([128, B], dt.bfloat16, tag="h0ts")
        transpose_to(h0ts[:], h0n[:], B, 128, id_bf)
        bh0_in = dram.tile([GS, B], dt.bfloat16, tag="bh0_in", bufs=3)
        nc.sync.dma_start(bh0_in[:], h0ts[:])
        bh0_out = dram.tile(
            [NCORES * GS, B], dt.bfloat16, tag="bh0_out", bufs=3, addr_space=SHARED
        )
        _allgather(nc, bh0_in[:], bh0_out[:])
        h0t_new = h0t_pp[(s + 1) % 2]
        nc.sync.dma_start(
            h0t_new[:], bh0_out[:].rearrange("(k p) b -> p k b", p=128)
        )

        # --- LSTM layer 1 ---
        ps_g1 = psB.tile([B, NG], dt.float32, tag="psB")
        nc.tensor.matmul(ps_g1[:], ones_bf[:, :B], bias_g1_sb[:], start=True, stop=False)
        for k in range(KH):
            nc.tensor.matmul(
                ps_g1[:], h1t_prev[:, k, :], whh1_sb[:, k, :],
                start=False, stop=False,
            )
        for k in range(KH):
            nc.tensor.matmul(
                ps_g1[:], h0t_new[:, k, :], wih1_sb[:, k, :],
                start=False, stop=(k == KH - 1),
            )
        c_l1_new = sbw.tile([B, GS], dt.float32, tag="c_l1n", bufs=2)
        h1n = lstm_gates(ps_g1, c_l1, c_l1_new, "l1")
        c_l1 = c_l1_new
        h1ts = sbw.tile([128, B], dt.bfloat16, tag="h1ts")
        transpose_to(h1ts[:], h1n[:], B, 128, id_bf)
        bh1_in = dram.tile([GS, B], dt.bfloat16, tag="bh1_in", bufs=3)
        nc.sync.dma_start(bh1_in[:], h1ts[:])
        bh1_out = hall_steps[s][:]
        _allgather(nc, bh1_in[:], bh1_out)
        nc.sync.dma_start(
            h1t_pp[(s + 1) % 2][:],
            bh1_out.rearrange("(k p) b -> p k b", p=128),
        )

    # close loop pools
    psC_cm.__exit__(None, None, None)
    psB_cm.__exit__(None, None, None)
    psA_cm.__exit__(None, None, None)
    sbw_cm.__exit__(None, None, None)
    loopers_cm.__exit__(None, None, None)

    # ---------------- phase 4: vocab projection + exp-sums ----------------
    p4_cm = tc.tile_pool(name="p4", bufs=3)
    p4 = p4_cm.__enter__()
    p4c_cm = tc.tile_pool(name="p4c", bufs=1)
    p4c = p4c_cm.__enter__()
    with tc.tile_pool(name="p4ps", bufs=4, space="PSUM") as p4ps:
        genb_sb = p4c.tile([1, VPC], dt.bfloat16, name="genb_sb")
        nc.sync.dma_start(genb_sb[:], di["genb_v"].ap())
        hhs = []
        for (m0, mr) in MTILES:
            s0 = m0 // B
            ns = mr // B
            hh = p4.tile([128, KH, 4, B], dt.bfloat16, tag=f"hh{m0}",
                         name=f"hh{m0}", bufs=1)
            for k in range(KH):
                for sl in range(ns):
                    nc.sync.dma_start(
                        hh[:, k, sl, :],
                        hall_steps[s0 + sl][:].rearrange(
                            "(k p) b -> k p b", p=128
                        )[k],
                    )
            hhs.append(hh)
        sparts_all = [
            p4.tile([128, len(VCH)], dt.float32, tag=f"sp{m0}",
                    name=f"sp{m0}", bufs=1)
            for (m0, mr) in MTILES
        ]
        logits_sb = [
            p4c.tile([128, VPC], dt.bfloat16, tag=f"lgs{m0}",
                     name=f"lgs{m0}", bufs=1)
            for (m0, mr) in MTILES
        ]
        for n, cw in enumerate(VCH):
            gw = p4.tile([128, KH, 512], dt.bfloat16, tag="gw")
            nc.sync.dma_start(
                gw[:, :, :cw],
                di["genw_t"].ap()[:, VOFF[n]:VOFF[n] + cw].rearrange(
                    "(k p) v -> p k v", p=128
                ),
            )
            for mi, (m0, mr) in enumerate(MTILES):
                ns = mr // B
                hh = hhs[mi]
                ps = p4ps.tile([128, 512], dt.float32, tag="p4p")
                nc.tensor.matmul(
                    ps[:mr, :cw], ones_bf[:, :mr],
                    genb_sb[:, VOFF[n]:VOFF[n] + cw], start=True, stop=False,
                )
                for k in range(KH):
                    nc.tensor.matmul(
                        ps[:mr, :cw],
                        hh[:, k, :ns, :],
                        gw[:, k, :cw],
                        start=False, stop=(k == KH - 1),
                    )
                scr = p4.tile([128, 512], dt.bfloat16, tag="scr")
                nc.scalar.activation(
                    scr[:mr, :cw], ps[:mr, :cw], AFT.Exp,
                    accum_out=sparts_all[mi][:mr, n:n + 1],
                )
                nc.vector.tensor_copy(
                    logits_sb[mi][:mr, VOFF[n]:VOFF[n] + cw], ps[:mr, :cw]
                )
        for mi, (m0, mr) in enumerate(MTILES):
            nc.vector.tensor_reduce(
                sume_all[:mr, mi:mi + 1], sparts_all[mi][:mr, :],
                axis=mybir.AxisListType.X, op=ALU.add,
            )

    # ---------------- phase 5: global logsumexp + subtract ----------------
    nm = len(MTILES)
    blse_in = dram.tile([128, nm], dt.float32, name="blse_in")
    blse_out = dram.tile([128, nm], dt.float32, name="blse_out", addr_space=SHARED)
    nc.sync.dma_start(blse_in[:], sume_all[:])
    _allreduce(nc, blse_in[:], blse_out[:])
    with tc.tile_pool(name="p5", bufs=2) as p5:
        sume_g = p5.tile([128, nm], dt.float32, name="sume_g", bufs=1)
        nc.sync.dma_start(sume_g[:], blse_out[:])
        lse = p5.tile([128, nm], dt.float32, name="lse", bufs=1)
        nc.scalar.activation(lse[:], sume_g[:], AFT.Ln)
        for mi, (m0, mr) in enumerate(MTILES):
            lpo = p5.tile([128, VPC], dt.float32, tag="lpo")
            nc.vector.tensor_scalar(
                lpo[:mr, :], logits_sb[mi][:mr, :], lse[:mr, mi:mi + 1],
                None, ALU.subtract,
            )
            nc.sync.dma_start(logp.ap()[m0:m0 + mr, :], lpo[:mr, :])
    p4c_cm.__exit__(None, None, None)
    p4_cm.__exit__(None, None, None)

    dram_cm.__exit__(None, None, None)
    glob_cm.__exit__(None, None, None)


def _prep_inputs(inputs):
    """Host-side sharding/layout prep. Returns list of per-core input dicts."""
    f32 = np.float32
    enc_out = np.asarray(inputs["enc_out"], f32)
    enc_h = np.asarray(inputs["enc_h"], f32)
    enc_c = np.asarray(inputs["enc_c"], f32)
    emb = np.asarray(inputs["embedding"], f32)
    attn_W = np.asarray(inputs["attn_W"], f32)
    attn_b = np.asarray(inputs["attn_b"], f32)
    vv = np.asarray(inputs["v"], f32)
    Wih0 = np.asarray(inputs["Wih0"], f32)
    Whh0 = np.asarray(inputs["Whh0"], f32)
    bih0 = np.asarray(inputs["bih0"], f32)
    bhh0 = np.asarray(inputs["bhh0"], f32)
    Wih1 = np.asarray(inputs["Wih1"], f32)
    Whh1 = np.asarray(inputs["Whh1"], f32)
    bih1 = np.asarray(inputs["bih1"], f32)
    bhh1 = np.asarray(inputs["bhh1"], f32)
    genW = np.asarray(inputs["genW"], f32)
    genb = np.asarray(inputs["genb"], f32)
    q = np.asarray(inputs["question"]).astype(np.int64)

    W1 = attn_W[:, :H]
    W2 = attn_W[:, H:]
    h0 = np.concatenate([enc_h[0], enc_h[1]], 1)  # (B, H) layer 0
    h1 = np.concatenate([enc_h[2], enc_h[3]], 1)  # layer 1
    c0 = np.concatenate([enc_c[0], enc_c[1]], 1)
    c1 = np.concatenate([enc_c[2], enc_c[3]], 1)
    qflat = q.T.reshape(NSAMP, 1).astype(np.int32)  # (s,b) order

    def bf(x):
        return np.ascontiguousarray(x).astype(BF)

    shared = {
        "w2t": bf(W2.T),
        "w1t": bf(W1.T),
        "attn_bias": bf(attn_b.reshape(1, H)),
        "vvec": bf(vv.reshape(H, 1)),
        "emb_tab": np.ascontiguousarray(emb),
        "qidx": qflat,
        "h0t_init": bf(h0.T),
        "h1t_init": bf(h1.T),
    }
    maps = []
    for c in range(NCORES):
        bs = slice(c * BPC, (c + 1) * BPC)
        rows = np.concatenate(
            [np.arange(g * H + c * GS, g * H + (c + 1) * GS) for g in range(4)]
        )
        wih0_s = Wih0[rows]  # (NG, E+H)
        wih0e = np.zeros((EP, NG), f32)
        wih0e[:E] = wih0_s[:, :E].T
        sel = np.zeros((B, BPC), f32)
        for j in range(BPC):
            sel[c * BPC + j, j] = 1.0
        vrows = slice(c * VPC, (c + 1) * VPC)
        m = dict(shared)
        m.update({
            "enc_nat": bf(enc_out[bs]),
            "enc_tr": bf(enc_out[bs].transpose(0, 2, 1)),
            "wih0e": bf(wih0e),
            "wih0c": bf(wih0_s[:, E:].T),
            "whh0": bf(Whh0[rows].T),
            "wih1": bf(Wih1[rows].T),
            "whh1": bf(Whh1[rows].T),
            "bias_g0": bf((bih0 + bhh0)[rows].reshape(1, NG)),
            "bias_g1": bf((bih1 + bhh1)[rows].reshape(1, NG)),
            "sel": bf(sel),
            "c0_l0": np.ascontiguousarray(c0[:, c * GS:(c + 1) * GS]),
            "c0_l1": np.ascontiguousarray(c1[:, c * GS:(c + 1) * GS]),
            "genw_t": bf(genW[vrows].T),
            "genb_v": bf(genb[vrows].reshape(1, VPC)),
        })
        maps.append(m)
    return maps


_CACHED = {}


def _get_compiled():
    if "nc" not in _CACHED:
        nc = bacc.Bacc(
            "TRN2", target_bir_lowering=False, debug=False,
            num_devices=1 if SIM1 else NCORES,
        )
        build(nc)
        nc.compile()
        _CACHED["nc"] = nc
    return _CACHED["nc"]


def run_cores(in_maps, **kw):
    nc = _get_compiled()
    return bass_utils.run_bass_kernel_spmd(nc, in_maps, list(range(NCORES)), **kw)


def kernel(**inputs):
    in_maps = _prep_inputs(inputs)
    res = run_cores(in_maps)
    parts = [res.results[c]["logp"] for c in range(NCORES)]
    full = np.concatenate(parts, axis=1)  # (NSAMP, V)
    out = full.reshape(S, B, V).transpose(1, 0, 2)
    return np.ascontiguousarray(out.astype(np.float32))
